# revision 12
# baseline (speedup 1.0000x reference)
"""Trainium2 Bass kernel for nn_AudioEncoder (vq_codebook).

Data-parallel over batch: 1 batch element per NeuronCore x 8 cores.

  - 7-layer strided conv stack as shift-and-matmul on the PE array.
  - All big matmuls run as fp32r (TF32-rate) 3-pass hi/lo split products:
    W.X ~= Wh.Xh + Wh.Xl + Wl.Xh with hi/lo tf32-mantissa-clean, giving
    fp32-level accuracy at ~3x the fp32 matmul rate.
  - GroupNorm: raw conv outputs stored to DRAM; per-channel sum/sumsq
    accumulated during PSUM eviction; 16-channel group aggregation via
    tiny indicator matmuls; normalization + GELU folded into a
    per-channel affine applied by the ScalarEngine when the next layer
    loads its input: gelu(scale_c * x + bias_c).
  - VQ: score = 2*f.c - |c|^2 via matmul (codebook pre-scaled by 2,
    -|c|^2 added at eviction), argmin via DVE MAX8/FIND_INDEX8,
    codebook gather via indirect DMA, then LayerNorm on device.
"""
import os
import sys

for _p in ("/opt/trn_rl_repo", "/root/.axon_site/_ro/trn_rl_repo"):
    if os.path.isdir(_p) and _p not in sys.path:
        sys.path.append(_p)

import numpy as np

import concourse.bass as bass
import concourse.tile as tile
from concourse import bacc, mybir

F32 = mybir.dt.float32
F32R = mybir.dt.float32r
U32 = mybir.dt.uint32
AF = mybir.ActivationFunctionType
ALU = mybir.AluOpType
AX = mybir.AxisListType

N_CORES = 8
NT = 512          # output columns per conv matmul tile
E = 1536          # embedding dim
NCODE = 1024      # codebook size
FB = 122          # frames per VQ block

_G = {}           # build-scoped shared tiles (sel, selT, eps, zero)
DBG_STOP = int(os.environ.get("K_DBG_STOP", "3"))
DBG_NOGATHER = os.environ.get("K_DBG_NOGATHER") == "1"
DBG_NOPB = os.environ.get("K_DBG_NOPB") == "1"


def layer_dims(L0):
    Ls = [L0, L0 // 5 + 1]
    for _ in range(6):
        Ls.append(Ls[-1] // 2 + 1)
    return Ls  # [L0, L1..L7]; S = Ls[7]


def _cdiv(a, b):
    return (a + b - 1) // b


def build_nc(L0):
    _G.clear()
    Ls = layer_dims(L0)
    S = Ls[7]
    nc = bacc.Bacc("TRN2", target_bir_lowering=False, debug=False,
                   enable_asserts=True, num_devices=N_CORES)
    t = {}

    def din(name, shape, dt=F32):
        t[name] = nc.dram_tensor(name, shape, dt, kind="ExternalInput").ap()

    din("x0h", [10, Ls[1]], F32R)
    din("x0l", [10, Ls[1]], F32R)
    din("w0h", [10, 256], F32R)
    din("w0l", [10, 256], F32R)
    din("b0", [256])
    din("g0", [256])
    din("be0", [256])
    for li in range(1, 7):
        cin = 256 if li == 1 else 512
        din(f"w{li}h", [10, cin, 512], F32R)
        din(f"w{li}l", [10, cin, 512], F32R)
        din(f"b{li}", [512])
        din(f"g{li}", [512])
        din(f"be{li}", [512])
    din("pwh", [512, E], F32R)
    din("pwl", [512, E], F32R)
    din("pb", [E])
    din("cbh", [E, NCODE], F32R)
    din("cbl", [E, NCODE], F32R)
    din("csqn", [1, NCODE])
    din("codebook", [NCODE, E])
    din("pos", [S, E])
    din("modality", [1, E])
    din("lng", [1, E])
    din("lnb", [1, E])
    din("sel", [128, 8])
    din("selT", [8, 128])

    acts = []
    for li in range(7):
        C = 256 if li == 0 else 512
        acts.append(nc.dram_tensor(f"act{li}", [C, Ls[li + 1]], F32,
                                   kind="Internal").ap())

    emb_out = nc.dram_tensor("emb", [S, E], F32, kind="ExternalOutput").ap()
    idx_out = nc.dram_tensor("idx", [S, 1], U32, kind="ExternalOutput").ap()

    build_program(nc, t, acts, emb_out, idx_out, Ls)
    nc.compile()
    return nc


def build_program(nc, t, acts, emb_out, idx_out, Ls):
    S = Ls[7]
    with tile.TileContext(nc) as tc:
        with tc.tile_pool(name="misc", bufs=1) as misc:
            sel = misc.tile([128, 8], F32, tag="sel")
            nc.sync.dma_start(sel[:], t["sel"][:])
            selT = misc.tile([8, 128], F32, tag="selT")
            nc.sync.dma_start(selT[:], t["selT"][:])
            eps = misc.tile([128, 1], F32, tag="eps")
            nc.vector.memset(eps[:], 1e-5)
            _G.update(sel=sel, selT=selT, eps=eps)
            # preload per-layer gn params (conv bias, gamma, beta) so the
            # layer-boundary stats->affine chain has no DMA latency in it
            for li in range(7):
                nb = 2 if li == 0 else 4
                for b in range(nb):
                    par = misc.tile([128, 4], F32, tag=f"par{li}_{b}",
                                    name=f"par{li}_{b}")
                    for j, nm in enumerate((f"b{li}", f"g{li}", f"be{li}")):
                        nc.sync.dma_start(
                            par[:, j:j + 1],
                            t[nm][b * 128:(b + 1) * 128].rearrange(
                                "(p f) -> p f", f=1))
                    _G[f"par{li}_{b}"] = par
            aff = {}
            stats = {}

            with tc.tile_pool(name="conv", bufs=1) as cp, \
                 tc.tile_pool(name="cpsum", bufs=1, space="PSUM") as pp:
                if DBG_STOP >= 1:
                    conv0(nc, t, misc, cp, pp, acts[0], Ls, aff, stats)
                for li in range(1, 7):
                    if DBG_STOP >= 1 and li <= int(os.environ.get("K_DBG_NLAYERS", "6")):
                        conv_layer(nc, t, misc, cp, pp, li, acts[li - 1],
                                   acts[li], Ls, aff, stats)

            if DBG_STOP < 2:
                with tc.tile_pool(name="dbg", bufs=1) as dp:
                    z = dp.tile([128, E], F32, tag="z")
                    nc.vector.memset(z[:], 0.0)
                    zi = dp.tile([128, 1], U32, tag="zi")
                    nc.vector.memset(zi[:], 0)
                    for f0 in range(0, S, 128):
                        m = min(128, S - f0)
                        nc.sync.dma_start(emb_out[f0:f0 + m, :], z[:m, :])
                        nc.sync.dma_start(idx_out[f0:f0 + m, :], zi[:m, :])
                return
            with tc.tile_pool(name="featp", bufs=1) as fpool:
                fh, fl = [], []
                for eb in range(12):
                    fh.append(fpool.tile([128, S + (S & 1)], F32R,
                                         tag=f"fh{eb}", name=f"fh{eb}"))
                    fl.append(fpool.tile([128, S + (S & 1)], F32R,
                                         tag=f"fl{eb}", name=f"fl{eb}"))
                with tc.tile_pool(name="projp", bufs=1) as jp, \
                     tc.tile_pool(name="jpsum", bufs=1, space="PSUM") as jpp:
                    proj(nc, t, jp, jpp, acts[6], Ls, aff, fh, fl)
                if DBG_STOP < 3:
                    with tc.tile_pool(name="dbg", bufs=1) as dp:
                        zi = dp.tile([128, 1], U32, tag="zi")
                        nc.vector.memset(zi[:], 0)
                        for f0 in range(0, S, 128):
                            m = min(128, S - f0)
                            nc.sync.dma_start(emb_out[f0:f0 + m, :],
                                              fh[0][:m, 0:E].bitcast(F32))
                            nc.sync.dma_start(idx_out[f0:f0 + m, :], zi[:m, :])
                    return
                with tc.tile_pool(name="vqp", bufs=1) as vp, \
                     tc.tile_pool(name="vpsum", bufs=1, space="PSUM") as vpp:
                    vq_ln(nc, t, vp, vpp, fh, fl, emb_out, idx_out, S)


def evict_with_stats(nc, cp, ps, w, s1col, s2col):
    out = cp.tile([128, NT], F32, tag="out", bufs=2, name="out")
    nc.scalar.activation(out[:, :w], ps[:, :w], AF.Identity,
                         accum_out=s1col)
    sq = cp.tile([128, NT], F32, tag="sq", bufs=1, name="sq")
    nc.scalar.activation(sq[:, :w], out[:, :w], AF.Square, accum_out=s2col)
    return out


def rstd_newton(nc, pool, v_in, v_out, parts, tagp=""):
    """v_out[:parts] = 1/sqrt(v_in[:parts]) with one Newton refinement."""
    s = pool.tile([128, 1], F32, tag=f"nw_s{tagp}", bufs=2, name="nws")
    nc.scalar.activation(s[:parts], v_in, AF.Sqrt)
    y0 = pool.tile([128, 1], F32, tag=f"nw_y0{tagp}", bufs=2, name="nwy")
    nc.vector.reciprocal(y0[:parts], s[:parts])
    u = pool.tile([128, 1], F32, tag=f"nw_u{tagp}", bufs=2, name="nwu")
    nc.vector.tensor_tensor(out=u[:parts], in0=v_in, in1=y0[:parts],
                            op=ALU.mult)
    nc.vector.tensor_tensor(out=u[:parts], in0=u[:parts], in1=y0[:parts],
                            op=ALU.mult)
    nc.vector.tensor_scalar(out=u[:parts], in0=u[:parts], scalar1=-0.5,
                            scalar2=1.5, op0=ALU.mult, op1=ALU.add)
    nc.vector.tensor_tensor(out=v_out, in0=y0[:parts], in1=u[:parts],
                            op=ALU.mult)


def gn_stats_to_affine(nc, misc, cp, pp, s1buf, s2buf, n_t, blk, Lout,
                       t, aff, par):
    st = cp.tile([128, 2], F32, tag="gn_st", bufs=2, name="gnst")
    nc.vector.reduce_sum(st[:, 0:1], s1buf[:, :n_t], axis=AX.X)
    nc.vector.reduce_sum(st[:, 1:2], s2buf[:, :n_t], axis=AX.X)
    w = cp.tile([128, 4], F32, tag="gn_w", bufs=2, name="gnw")
    st2 = cp.tile([128, 2], F32, tag="gn_st2", bufs=2, name="gnst2")
    # S1' = S1 + L*b
    nc.vector.tensor_scalar(out=w[:, 1:2], in0=par[:, 0:1],
                            scalar1=float(Lout), scalar2=None, op0=ALU.mult)
    nc.vector.tensor_tensor(out=st2[:, 0:1], in0=st[:, 0:1], in1=w[:, 1:2],
                            op=ALU.add)
    # S2' = S2 + 2*b*S1 + L*b^2
    nc.vector.tensor_tensor(out=w[:, 0:1], in0=st[:, 0:1], in1=par[:, 0:1],
                            op=ALU.mult)
    nc.vector.tensor_scalar(out=w[:, 3:4], in0=w[:, 0:1], scalar1=2.0,
                            scalar2=None, op0=ALU.mult)
    nc.vector.tensor_tensor(out=w[:, 2:3], in0=w[:, 1:2], in1=par[:, 0:1],
                            op=ALU.mult)
    nc.vector.tensor_tensor(out=w[:, 2:3], in0=w[:, 2:3], in1=w[:, 3:4],
                            op=ALU.add)
    nc.vector.tensor_tensor(out=st2[:, 1:2], in0=st[:, 1:2], in1=w[:, 2:3],
                            op=ALU.add)
    # group aggregate [8, 2]
    gagg = pp.tile([8, 2], F32, tag="gn_ps", bufs=2, name="gnagg")
    nc.tensor.matmul(gagg[:], _G["sel"][:], st2[:], start=True, stop=True)
    ga = cp.tile([8, 4], F32, tag="gn_ga", bufs=2, name="gnga")
    inv = 1.0 / (16.0 * Lout)
    nc.scalar.activation(ga[:, 0:2], gagg[:], AF.Identity, scale=inv)
    nc.vector.tensor_tensor(out=ga[:, 2:3], in0=ga[:, 0:1], in1=ga[:, 0:1],
                            op=ALU.mult)
    nc.vector.tensor_tensor(out=ga[:, 2:3], in0=ga[:, 1:2], in1=ga[:, 2:3],
                            op=ALU.subtract)
    nc.vector.tensor_tensor(out=ga[:, 2:3], in0=ga[:, 2:3],
                            in1=_G["eps"][0:8, 0:1], op=ALU.add)
    rstd_newton(nc, cp, ga[:, 2:3], ga[:, 3:4], 8, tagp="g")
    mr = cp.tile([8, 2], F32, tag="gn_mr", bufs=2, name="gnmr")
    nc.vector.tensor_copy(mr[:, 0:1], ga[:, 0:1])
    nc.vector.tensor_copy(mr[:, 1:2], ga[:, 3:4])
    gex = pp.tile([128, 2], F32, tag="gn_ps", bufs=2, name="gnexp")
    nc.tensor.matmul(gex[:], _G["selT"][:], mr[:], start=True, stop=True)
    exs = cp.tile([128, 2], F32, tag="gn_exs", bufs=2, name="gnexs")
    nc.scalar.activation(exs[:], gex[:], AF.Identity)
    # scale_c = gamma*rstd ; bias_c = beta + scale_c*(b - mean)
    a = misc.tile([128, 2], F32, tag=f"aff{blk}", name=f"aff{blk}")
    nc.vector.tensor_tensor(out=a[:, 0:1], in0=par[:, 1:2], in1=exs[:, 1:2],
                            op=ALU.mult)
    tmp = cp.tile([128, 1], F32, tag="gn_tmp", bufs=2, name="gntmp")
    nc.vector.tensor_tensor(out=tmp[:], in0=par[:, 0:1], in1=exs[:, 0:1],
                            op=ALU.subtract)
    nc.vector.tensor_tensor(out=tmp[:], in0=tmp[:], in1=a[:, 0:1],
                            op=ALU.mult)
    nc.vector.tensor_tensor(out=a[:, 1:2], in0=par[:, 2:3], in1=tmp[:],
                            op=ALU.add)
    aff[blk] = a


def conv0(nc, t, misc, cp, pp, a0, Ls, aff, stats):
    L1 = Ls[1]
    n_t = _cdiv(L1, NT)
    wh = cp.tile([10, 256], F32R, tag="w0h", name="w0h")
    nc.sync.dma_start(wh[:], t["w0h"][:])
    wl = cp.tile([10, 256], F32R, tag="w0l", name="w0l")
    nc.sync.dma_start(wl[:], t["w0l"][:])
    s1 = {}
    s2 = {}
    for b in range(2):
        s1[b] = cp.tile([128, n_t], F32, tag=f"s1_{b}", bufs=2, name=f"s1c0{b}")
        s2[b] = cp.tile([128, n_t], F32, tag=f"s2_{b}", bufs=2, name=f"s2c0{b}")
    XG = 3
    for tg in range(0, n_t, XG):
        g0c = tg * NT
        gcols = min(XG * NT, L1 - g0c)
        gcols2 = gcols + (gcols & 1)
        xh = cp.tile([10, XG * NT], F32R, tag="x0h", bufs=2, name="x0h")
        nc.sync.dma_start(xh[:, :gcols], t["x0h"][:, g0c:g0c + gcols])
        xl = cp.tile([10, XG * NT], F32R, tag="x0l", bufs=2, name="x0l")
        nc.sync.dma_start(xl[:, :gcols], t["x0l"][:, g0c:g0c + gcols])
        if gcols2 > gcols:
            nc.vector.memset(xh[:, gcols:gcols2].bitcast(F32), 0.0)
            nc.vector.memset(xl[:, gcols:gcols2].bitcast(F32), 0.0)
        for ti in range(tg, min(tg + XG, n_t)):
            n0 = ti * NT
            w = min(NT, L1 - n0)
            w2 = w + (w & 1)
            c0 = n0 - g0c
            for b in range(2):
                ps = pp.tile([128, NT], F32, tag=f"mm{b}", bufs=3,
                             name="psc0")
                co = slice(b * 128, (b + 1) * 128)
                nc.tensor.matmul(ps[:, :w2], wh[:, co], xh[:, c0:c0 + w2],
                                 start=True, stop=False)
                nc.tensor.matmul(ps[:, :w2], wh[:, co], xl[:, c0:c0 + w2],
                                 start=False, stop=False)
                nc.tensor.matmul(ps[:, :w2], wl[:, co], xh[:, c0:c0 + w2],
                                 start=False, stop=True)
                out = evict_with_stats(nc, cp, ps, w, s1[b][:, ti:ti + 1],
                                       s2[b][:, ti:ti + 1])
                nc.sync.dma_start(a0[b * 128:(b + 1) * 128, n0:n0 + w],
                                  out[:, :w])
    for b in range(2):
        gn_stats_to_affine(nc, misc, cp, pp, s1[b], s2[b], n_t, b, L1,
                           t, aff, _G[f"par0_{b}"])


def conv_layer(nc, t, misc, cp, pp, li, a_in, a_out, Ls, aff, stats):
    Lin, Lout = Ls[li], Ls[li + 1]
    cin = 256 if li == 1 else 512
    n_ci = cin // 128
    n_t = _cdiv(Lout, NT)
    WMAX = 2 * NT + 10
    # snapshot input affines (written by previous layer) before this layer
    # overwrites aff[blk] at its own finalize
    in_aff = [aff[ci] for ci in range(n_ci)]
    for pas in range(2):
        blocks = [pas * 2, pas * 2 + 1]
        whl = {}
        for b in blocks:
            for hl in "hl":
                wt = cp.tile([128, n_ci * 10 * 128], F32R,
                             tag=f"w{hl}{b % 2}", name=f"w{hl}{b % 2}")
                for ci in range(n_ci):
                    for k in range(10):
                        co = ((ci * 10) + k) * 128
                        nc.sync.dma_start(
                            wt[:, co:co + 128],
                            t[f"w{li}{hl}"][k, ci * 128:(ci + 1) * 128,
                                            b * 128:(b + 1) * 128])
                whl[(b, hl)] = wt
        sb = {}
        for b in blocks:
            sb[("s1", b)] = cp.tile([128, n_t], F32, tag=f"s1_{b % 2}",
                                    bufs=2, name=f"s1_{li}_{b}")
            sb[("s2", b)] = cp.tile([128, n_t], F32, tag=f"s2_{b % 2}",
                                    bufs=2, name=f"s2_{li}_{b}")
        for ti in range(n_t):
            n0 = ti * NT
            w = min(NT, Lout - n0)
            w2 = w + (w & 1)
            ilo = 2 * n0 - 5
            width2 = 2 * w2 + 8
            vlo, vhi = max(ilo, 0), min(ilo + width2 - 1, Lin - 1)
            ds, de = vlo - ilo, vhi - ilo + 1
            his, los = [], []
            PW = NT + 4
            for ci in range(n_ci):
                raw = cp.tile([128, WMAX], F32, tag="raw", bufs=3, name="raw")
                nc.sync.dma_start(raw[:, ds:de],
                                  a_in[ci * 128:(ci + 1) * 128, vlo:vhi + 1])
                ac = in_aff[ci]
                # deinterleave into even/odd input phases so conv matmul rhs
                # APs are stride-1 (stride-2 fp32r matmul runs at half rate):
                # even phase e0 = n0-2 at raw col 2r+1; odd phase o0 = n0-3
                # at raw col 2r.
                phs = []
                for po, (j0, rs, re) in enumerate((
                        (1, ds // 2, de // 2),
                        (0, (ds + 1) // 2, (de + 1) // 2))):
                    gph = cp.tile([128, PW], F32, tag=f"g{po}", bufs=2,
                                  name=f"g{po}")
                    nc.scalar.activation(gph[:, rs:re],
                                         raw[:, j0 + 2 * rs:j0 + 2 * re:2],
                                         AF.Gelu, bias=ac[:, 1:2],
                                         scale=ac[:, 0:1])
                    hi = cp.tile([128, PW], F32R, tag=f"xh{po}_{ci}", bufs=2,
                                 name=f"xh{po}_{ci}")
                    lo = cp.tile([128, PW], F32R, tag=f"xl{po}_{ci}", bufs=2,
                                 name=f"xl{po}_{ci}")
                    nc.vector.tensor_copy(hi[:, rs:re], gph[:, rs:re])
                    nc.vector.tensor_tensor(out=lo[:, rs:re],
                                            in0=gph[:, rs:re],
                                            in1=hi[:, rs:re].bitcast(F32),
                                            op=ALU.subtract)
                    wph = w2 + 4
                    if rs > 0:
                        nc.vector.memset(hi[:, 0:rs].bitcast(F32), 0.0)
                        nc.vector.memset(lo[:, 0:rs].bitcast(F32), 0.0)
                    if re < wph:
                        nc.vector.memset(hi[:, re:wph].bitcast(F32), 0.0)
                        nc.vector.memset(lo[:, re:wph].bitcast(F32), 0.0)
                    phs.append((hi, lo))
                his.append((phs[0][0], phs[1][0]))   # (even, odd) hi
                los.append((phs[0][1], phs[1][1]))   # (even, odd) lo
            for b in blocks:
                ps = pp.tile([128, NT], F32, tag=f"mm{b % 2}", bufs=3,
                             name="psc")
                first = True
                for ci in range(n_ci):
                    wh_ci = whl[(b, 'h')]
                    wl_ci = whl[(b, 'l')]
                    for k in range(10):
                        co = ((ci * 10) + k) * 128
                        if k % 2 == 1:
                            st = (k - 1) // 2
                            rh = his[ci][0][:, st:st + w2]
                            rl = los[ci][0][:, st:st + w2]
                        else:
                            st = k // 2
                            rh = his[ci][1][:, st:st + w2]
                            rl = los[ci][1][:, st:st + w2]
                        last = (ci == n_ci - 1 and k == 9)
                        nc.tensor.matmul(ps[:, :w2], wh_ci[:, co:co + 128], rh,
                                         start=first, stop=False)
                        first = False
                        nc.tensor.matmul(ps[:, :w2], wh_ci[:, co:co + 128], rl,
                                         start=False, stop=False)
                        nc.tensor.matmul(ps[:, :w2], wl_ci[:, co:co + 128], rh,
                                         start=False, stop=last)
                out = evict_with_stats(nc, cp, ps, w,
                                       sb[("s1", b)][:, ti:ti + 1],
                                       sb[("s2", b)][:, ti:ti + 1])
                nc.sync.dma_start(a_out[b * 128:(b + 1) * 128, n0:n0 + w],
                                  out[:, :w])
        for b in blocks:
            gn_stats_to_affine(nc, misc, cp, pp, sb[("s1", b)],
                               sb[("s2", b)], n_t, b, Lout, t, aff,
                               _G[f"par{li}_{b}"])


def proj(nc, t, jp, jpp, a6, Ls, aff, fh, fl):
    S = Ls[7]
    S2 = S + (S & 1)
    a6h, a6l = [], []
    for ci in range(4):
        raw = jp.tile([128, S], F32, tag="a6raw", bufs=2, name="a6raw")
        nc.sync.dma_start(raw[:], a6[ci * 128:(ci + 1) * 128, :])
        g = jp.tile([128, S], F32, tag="a6g", bufs=2, name="a6g")
        ac = aff[ci]
        nc.scalar.activation(g[:], raw[:], AF.Gelu, bias=ac[:, 1:2],
                             scale=ac[:, 0:1])
        hi = jp.tile([128, S2], F32R, tag=f"a6h{ci}", name=f"a6h{ci}")
        lo = jp.tile([128, S2], F32R, tag=f"a6l{ci}", name=f"a6l{ci}")
        nc.vector.tensor_copy(hi[:, :S], g[:])
        nc.vector.tensor_tensor(out=lo[:, :S], in0=g[:],
                                in1=hi[:, :S].bitcast(F32), op=ALU.subtract)
        if S2 > S:
            nc.vector.memset(hi[:, S:S2].bitcast(F32), 0.0)
            nc.vector.memset(lo[:, S:S2].bitcast(F32), 0.0)
        a6h.append(hi)
        a6l.append(lo)
    n_t = _cdiv(S, NT)
    for eb in range(12):
        pwh = jp.tile([128, 4 * 128], F32R, tag="pwh", bufs=2, name="pwh")
        pwl = jp.tile([128, 4 * 128], F32R, tag="pwl", bufs=2, name="pwl")
        for ci in range(4):
            nc.sync.dma_start(pwh[:, ci * 128:(ci + 1) * 128],
                              t["pwh"][ci * 128:(ci + 1) * 128,
                                       eb * 128:(eb + 1) * 128])
            nc.sync.dma_start(pwl[:, ci * 128:(ci + 1) * 128],
                              t["pwl"][ci * 128:(ci + 1) * 128,
                                       eb * 128:(eb + 1) * 128])
        pbias = jp.tile([128, 1], F32, tag="pbias", bufs=2, name="pbias")
        nc.sync.dma_start(
            pbias[:],
            t["pb"][eb * 128:(eb + 1) * 128].rearrange("(p f) -> p f", f=1))
        fraw = jp.tile([128, S], F32, tag="fraw", bufs=2, name="fraw")
        for ti in range(n_t):
            n0 = ti * NT
            w = min(NT, S - n0)
            w2 = w + (w & 1)
            ps = jpp.tile([128, NT], F32, tag=f"pp{ti % 2}", bufs=2,
                          name="psp")
            first = True
            for ci in range(4):
                cw = slice(ci * 128, (ci + 1) * 128)
                last = ci == 3
                nc.tensor.matmul(ps[:, :w2], pwh[:, cw],
                                 a6h[ci][:, n0:n0 + w2],
                                 start=first, stop=False)
                first = False
                nc.tensor.matmul(ps[:, :w2], pwh[:, cw],
                                 a6l[ci][:, n0:n0 + w2],
                                 start=False, stop=False)
                nc.tensor.matmul(ps[:, :w2], pwl[:, cw],
                                 a6h[ci][:, n0:n0 + w2],
                                 start=False, stop=last)
            # (eviction below only covers the true w columns)
            nc.vector.tensor_scalar(out=fraw[:, n0:n0 + w], in0=ps[:, :w],
                                    scalar1=pbias[:, 0:1], scalar2=None,
                                    op0=ALU.add)
        nc.vector.tensor_copy(fh[eb][:, :S], fraw[:])
        nc.vector.tensor_tensor(out=fl[eb][:, :S], in0=fraw[:],
                                in1=fh[eb][:, :S].bitcast(F32),
                                op=ALU.subtract)
        if S2 > S:
            nc.vector.memset(fh[eb][:, S:S2].bitcast(F32), 0.0)
            nc.vector.memset(fl[eb][:, S:S2].bitcast(F32), 0.0)


def vq_ln(nc, t, vp, vpp, fh, fl, emb_out, idx_out, S):
    S2 = S + (S & 1)
    csq = vp.tile([1, NCODE], F32, tag="csq1", name="csq1")
    nc.sync.dma_start(csq[:], t["csqn"][:])
    csqb = vp.tile([128, NCODE], F32, tag="csqb", name="csqb")
    if DBG_NOPB:
        nc.vector.memset(csqb[:], 0.0)
    else:
        nc.gpsimd.partition_broadcast(csqb[:], csq[:])
    modb = vp.tile([128, E], F32, tag="modb", name="modb")
    lngb = vp.tile([128, E], F32, tag="lngb", name="lngb")
    lnbb = vp.tile([128, E], F32, tag="lnbb", name="lnbb")
    for bt, nm in ((modb, "modality"), (lngb, "lng"), (lnbb, "lnb")):
        if DBG_NOPB:
            nc.vector.memset(bt[:], 0.0)
            continue
        m1 = vp.tile([1, E], F32, tag="m1", bufs=2, name="m1")
        nc.sync.dma_start(m1[:], t[nm][:])
        nc.gpsimd.partition_broadcast(bt[:], m1[:])

    nfb = _cdiv(S, FB)
    for fbg in range(0, nfb, 4):
        fbs = list(range(fbg, min(fbg + 4, nfb)))
        pss = {}
        for fb in fbs:
            pss[fb] = vpp.tile([128, NCODE], F32, tag=f"vq{fb % 4}", bufs=1,
                               name=f"psv{fb % 4}")
        for eb in range(12):
            ch = vp.tile([128, NCODE], F32R, tag="cbh", bufs=2, name="cbh")
            cl = vp.tile([128, NCODE], F32R, tag="cbl", bufs=2, name="cbl")
            nc.sync.dma_start(ch[:], t["cbh"][eb * 128:(eb + 1) * 128, :])
            nc.sync.dma_start(cl[:], t["cbl"][eb * 128:(eb + 1) * 128, :])
            for fb in fbs:
                f0 = fb * FB
                m = min(FB, S - f0)
                m2 = m + (m & 1)
                ps = pss[fb]
                for half in range(2):
                    cs = slice(half * 512, (half + 1) * 512)
                    nc.tensor.matmul(ps[:m2, cs], fh[eb][:, f0:f0 + m2],
                                     ch[:, cs], start=(eb == 0), stop=False)
                    nc.tensor.matmul(ps[:m2, cs], fh[eb][:, f0:f0 + m2],
                                     cl[:, cs], start=False, stop=False)
                    nc.tensor.matmul(ps[:m2, cs], fl[eb][:, f0:f0 + m2],
                                     ch[:, cs], start=False,
                                     stop=(eb == 11))
        vq_post(nc, t, vp, fh, fl, emb_out, idx_out, S, fbs, pss,
                csqb, modb, lngb, lnbb)


def vq_post(nc, t, vp, fh, fl, emb_out, idx_out, S, fbs, pss,
            csqb, modb, lngb, lnbb):
    for fb in fbs:
        f0 = fb * FB
        m = min(FB, S - f0)
        ps = pss[fb]
        score = vp.tile([128, NCODE], F32, tag="score", bufs=3, name="score")
        nc.vector.tensor_tensor(out=score[:m, :], in0=ps[:m, :],
                                in1=csqb[:m, :], op=ALU.add)
        mx8 = vp.tile([128, 8], F32, tag="mx8", bufs=3, name="mx8")
        ix8 = vp.tile([128, 8], U32, tag="ix8", bufs=3, name="ix8")
        nc.vector.max(mx8[:m, :], score[:m, :])
        nc.vector.max_index(ix8[:m, :], mx8[:m, :], score[:m, :])
        nc.sync.dma_start(idx_out[f0:f0 + m, :], ix8[:m, 0:1])
        q = vp.tile([128, E], F32, tag="q", bufs=3, name="q")
        if DBG_NOGATHER:
            nc.sync.dma_start(q[:m, :], t["codebook"][f0:f0 + m, :])
        else:
            nc.gpsimd.indirect_dma_start(
                out=q[:m, :], out_offset=None, in_=t["codebook"][:],
                in_offset=bass.IndirectOffsetOnAxis(ap=ix8[:m, 0:1], axis=0))
        pos = vp.tile([128, E], F32, tag="pos", bufs=2, name="pos")
        nc.sync.dma_start(pos[:m, :], t["pos"][f0:f0 + m, :])
        nc.vector.tensor_tensor(out=q[:m, :], in0=q[:m, :], in1=pos[:m, :],
                                op=ALU.add)
        nc.vector.tensor_tensor(out=q[:m, :], in0=q[:m, :], in1=modb[:m, :],
                                op=ALU.add)
        # LayerNorm over E
        stat = vp.tile([128, 4], F32, tag="lnstat", bufs=3, name="lnstat")
        nc.vector.reduce_sum(stat[:m, 0:1], q[:m, :], axis=AX.X)
        nc.vector.tensor_scalar(out=stat[:m, 1:2], in0=stat[:m, 0:1],
                                scalar1=1.0 / E, scalar2=None, op0=ALU.mult)
        nc.vector.tensor_scalar(out=q[:m, :], in0=q[:m, :],
                                scalar1=stat[:m, 1:2], scalar2=None,
                                op0=ALU.subtract)
        sq = vp.tile([128, E], F32, tag="lnsq", bufs=2, name="lnsq")
        nc.scalar.activation(sq[:m, :], q[:m, :], AF.Square,
                             accum_out=stat[:m, 2:3])
        v = vp.tile([128, 2], F32, tag="lnv", bufs=3, name="lnv")
        nc.vector.tensor_scalar(out=v[:m, 0:1], in0=stat[:m, 2:3],
                                scalar1=1.0 / E, scalar2=None, op0=ALU.mult)
        nc.vector.tensor_tensor(out=v[:m, 0:1], in0=v[:m, 0:1],
                                in1=_G["eps"][:m, 0:1], op=ALU.add)
        rstd_newton(nc, vp, v[:m, 0:1], v[:m, 1:2], m, tagp="v")
        nc.vector.tensor_scalar(out=q[:m, :], in0=q[:m, :],
                                scalar1=v[:m, 1:2], scalar2=None,
                                op0=ALU.mult)
        nc.vector.tensor_tensor(out=q[:m, :], in0=q[:m, :], in1=lngb[:m, :],
                                op=ALU.mult)
        nc.vector.tensor_tensor(out=q[:m, :], in0=q[:m, :], in1=lnbb[:m, :],
                                op=ALU.add)
        nc.sync.dma_start(emb_out[f0:f0 + m, :], q[:m, :])


# ---------------------------------------------------------------- host side

def tf32_rne(x):
    b = np.ascontiguousarray(x, dtype=np.float32).view(np.uint32)
    keep = np.uint32(13)
    rb = np.uint32(1 << 12)
    low = b & np.uint32((1 << 13) - 1)
    b2 = b & ~np.uint32((1 << 13) - 1)
    inc = (low > rb) | ((low == rb) & (((b2 >> keep) & np.uint32(1)) == 1))
    return (b2 + (inc.astype(np.uint32) << keep)).view(np.float32)


def split_hl(x):
    x = np.ascontiguousarray(x, np.float32)
    h = tf32_rne(x)
    l = tf32_rne(x - h)
    return h, l


def prep_shared(inputs, L0):
    Ls = layer_dims(L0)
    S = Ls[7]
    m = {}
    w0 = np.asarray(inputs["conv_w0"], np.float32)[:, 0, :].T
    m["w0h"], m["w0l"] = split_hl(w0)
    m["b0"] = np.ascontiguousarray(inputs["conv_b0"], np.float32)
    m["g0"] = np.ascontiguousarray(inputs["gn_g0"], np.float32)
    m["be0"] = np.ascontiguousarray(inputs["gn_b0"], np.float32)
    w1 = np.asarray(inputs["conv_w1"], np.float32).transpose(2, 1, 0)
    m["w1h"], m["w1l"] = split_hl(w1)
    m["b1"] = np.ascontiguousarray(inputs["conv_b1"], np.float32)
    gn_gr = np.asarray(inputs["gn_gr"], np.float32)
    gn_br = np.asarray(inputs["gn_br"], np.float32)
    m["g1"] = np.ascontiguousarray(gn_gr[0])
    m["be1"] = np.ascontiguousarray(gn_br[0])
    wr = np.asarray(inputs["conv_wr"], np.float32)
    br = np.asarray(inputs["conv_br"], np.float32)
    for i in range(5):
        m[f"w{i + 2}h"], m[f"w{i + 2}l"] = split_hl(wr[i].transpose(2, 1, 0))
        m[f"b{i + 2}"] = np.ascontiguousarray(br[i])
        m[f"g{i + 2}"] = np.ascontiguousarray(gn_gr[i + 1])
        m[f"be{i + 2}"] = np.ascontiguousarray(gn_br[i + 1])
    pw = np.asarray(inputs["proj_w"], np.float32).T
    m["pwh"], m["pwl"] = split_hl(pw)
    m["pb"] = np.ascontiguousarray(inputs["proj_b"], np.float32)
    cb = np.ascontiguousarray(inputs["codebook"], np.float32)
    m["cbh"], m["cbl"] = split_hl((2.0 * cb).T)
    m["csqn"] = np.ascontiguousarray(
        -np.sum(cb.astype(np.float64) ** 2, axis=1).astype(np.float32)[None, :])
    m["codebook"] = cb
    m["pos"] = np.ascontiguousarray(
        np.asarray(inputs["pos_enc"], np.float32)[0, :S, :])
    m["modality"] = np.ascontiguousarray(
        np.asarray(inputs["modality"], np.float32).reshape(1, E))
    m["lng"] = np.ascontiguousarray(
        np.asarray(inputs["ln_g"], np.float32).reshape(1, E))
    m["lnb"] = np.ascontiguousarray(
        np.asarray(inputs["ln_b"], np.float32).reshape(1, E))
    sel = np.zeros((128, 8), np.float32)
    sel[np.arange(128), np.arange(128) // 16] = 1.0
    m["sel"] = sel
    m["selT"] = np.ascontiguousarray(sel.T)
    return m


def prep_waveform(wav, L0):
    L1 = layer_dims(L0)[1]
    pad = np.zeros(L0 + 10, np.float32)
    pad[5:5 + L0] = wav
    idx = np.arange(L1)[None, :] * 5 + np.arange(10)[:, None]
    h, l = split_hl(pad[idx])
    return {"x0h": h, "x0l": l}


_NC_CACHE = {}


def get_nc(L0):
    if L0 not in _NC_CACHE:
        _NC_CACHE[L0] = build_nc(L0)
    return _NC_CACHE[L0]


def make_in_maps(inputs):
    wav = np.asarray(inputs["waveform"], np.float32)
    B, L0 = wav.shape
    shared = prep_shared(inputs, L0)
    in_maps = []
    for b in range(B):
        im = dict(shared)
        im.update(prep_waveform(wav[b], L0))
        in_maps.append(im)
    return in_maps


def kernel(**inputs):
    from concourse.bass_utils import run_bass_kernel_spmd

    wav = np.asarray(inputs["waveform"], np.float32)
    B, L0 = wav.shape
    assert B == N_CORES
    S = layer_dims(L0)[7]
    nc = get_nc(L0)
    in_maps = make_in_maps(inputs)
    res = run_bass_kernel_spmd(nc, in_maps, list(range(N_CORES))).results
    emb = np.stack([res[b]["emb"] for b in range(B)])
    idx = np.stack([res[b]["idx"][:, 0] for b in range(B)]).astype(np.int32)
    mask = np.ones((B, S), np.int32)
    return emb, mask, idx


# revision 16
# speedup vs baseline: 1.0992x; 1.0992x over previous
"""Trainium2 Bass kernel for nn_AudioEncoder (vq_codebook).

Data-parallel over batch: 1 batch element per NeuronCore x 8 cores.

  - 7-layer strided conv stack as shift-and-matmul on the PE array.
  - All big matmuls run as fp32r (TF32-rate) 3-pass hi/lo split products:
    W.X ~= Wh.Xh + Wh.Xl + Wl.Xh with hi/lo tf32-mantissa-clean, giving
    fp32-level accuracy at ~3x the fp32 matmul rate.
  - GroupNorm: raw conv outputs stored to DRAM; per-channel sum/sumsq
    accumulated during PSUM eviction; 16-channel group aggregation via
    tiny indicator matmuls; normalization + GELU folded into a
    per-channel affine applied by the ScalarEngine when the next layer
    loads its input: gelu(scale_c * x + bias_c).
  - VQ: score = 2*f.c - |c|^2 via matmul (codebook pre-scaled by 2,
    -|c|^2 added at eviction), argmin via DVE MAX8/FIND_INDEX8,
    codebook gather via indirect DMA, then LayerNorm on device.
"""
import os
import sys

for _p in ("/opt/trn_rl_repo", "/root/.axon_site/_ro/trn_rl_repo"):
    if os.path.isdir(_p) and _p not in sys.path:
        sys.path.append(_p)

import numpy as np

import concourse.bass as bass
import concourse.tile as tile
from concourse import bacc, mybir

F32 = mybir.dt.float32
F32R = mybir.dt.float32r
U32 = mybir.dt.uint32
AF = mybir.ActivationFunctionType
ALU = mybir.AluOpType
AX = mybir.AxisListType

N_CORES = 8
NT = 512          # output columns per conv matmul tile
E = 1536          # embedding dim
NCODE = 1024      # codebook size
FB = 122          # frames per VQ block

_G = {}           # build-scoped shared tiles (sel, selT, eps, zero)
DBG_STOP = int(os.environ.get("K_DBG_STOP", "3"))
DBG_NOGATHER = os.environ.get("K_DBG_NOGATHER") == "1"
DBG_NOPB = os.environ.get("K_DBG_NOPB") == "1"


def layer_dims(L0):
    Ls = [L0, L0 // 5 + 1]
    for _ in range(6):
        Ls.append(Ls[-1] // 2 + 1)
    return Ls  # [L0, L1..L7]; S = Ls[7]


def _cdiv(a, b):
    return (a + b - 1) // b


def build_nc(L0):
    _G.clear()
    Ls = layer_dims(L0)
    S = Ls[7]
    nc = bacc.Bacc("TRN2", target_bir_lowering=False, debug=False,
                   enable_asserts=True, num_devices=N_CORES)
    t = {}

    def din(name, shape, dt=F32):
        t[name] = nc.dram_tensor(name, shape, dt, kind="ExternalInput").ap()

    din("x0h", [10, Ls[1]], F32R)
    din("x0l", [10, Ls[1]], F32R)
    din("w0h", [10, 256], F32R)
    din("w0l", [10, 256], F32R)
    din("b0", [256])
    din("g0", [256])
    din("be0", [256])
    for li in range(1, 7):
        cin = 256 if li == 1 else 512
        din(f"w{li}h", [10, cin, 512], F32R)
        din(f"w{li}l", [10, cin, 512], F32R)
        din(f"b{li}", [512])
        din(f"g{li}", [512])
        din(f"be{li}", [512])
    din("pwh", [512, E], F32R)
    din("pwl", [512, E], F32R)
    din("pb", [E])
    din("cbh", [E, NCODE], F32R)
    din("cbl", [E, NCODE], F32R)
    din("csqn", [1, NCODE])
    din("codebook", [NCODE, E])
    din("pos", [S, E])
    din("modality", [1, E])
    din("lng", [1, E])
    din("lnb", [1, E])
    din("sel", [128, 8])
    din("selT", [8, 128])

    acts = []
    for li in range(7):
        C = 256 if li == 0 else 512
        acts.append(nc.dram_tensor(f"act{li}", [C, Ls[li + 1]], F32,
                                   kind="Internal").ap())

    emb_out = nc.dram_tensor("emb", [S, E], F32, kind="ExternalOutput").ap()
    idx_out = nc.dram_tensor("idx", [S, 1], U32, kind="ExternalOutput").ap()

    build_program(nc, t, acts, emb_out, idx_out, Ls)
    nc.compile()
    return nc


def build_program(nc, t, acts, emb_out, idx_out, Ls):
    S = Ls[7]
    with tile.TileContext(nc) as tc:
        with tc.tile_pool(name="misc", bufs=1) as misc:
            sel = misc.tile([128, 8], F32, tag="sel")
            nc.sync.dma_start(sel[:], t["sel"][:])
            selT = misc.tile([8, 128], F32, tag="selT")
            nc.sync.dma_start(selT[:], t["selT"][:])
            eps = misc.tile([128, 8], F32, tag="eps")
            nc.vector.memset(eps[:], 1e-5)
            _G.update(sel=sel, selT=selT, eps=eps)
            # preload per-layer gn params (conv bias, gamma, beta) so the
            # layer-boundary stats->affine chain has no DMA latency in it
            for li in range(7):
                nb = 2 if li == 0 else 4
                for b in range(nb):
                    par = misc.tile([128, 8], F32, tag=f"par{li}_{b}",
                                    name=f"par{li}_{b}")
                    for j, nm in enumerate((f"b{li}", f"g{li}", f"be{li}")):
                        nc.sync.dma_start(
                            par[:, j:j + 1],
                            t[nm][b * 128:(b + 1) * 128].rearrange(
                                "(p f) -> p f", f=1))
                    _G[f"par{li}_{b}"] = par
            aff = {}
            stats = {}

            with tc.tile_pool(name="conv", bufs=1) as cp, \
                 tc.tile_pool(name="cpsum", bufs=1, space="PSUM") as pp:
                if DBG_STOP >= 1:
                    conv0(nc, t, misc, cp, pp, acts[0], Ls, aff, stats)
                for li in range(1, 7):
                    if DBG_STOP >= 1 and li <= int(os.environ.get("K_DBG_NLAYERS", "6")):
                        conv_layer(nc, t, misc, cp, pp, li, acts[li - 1],
                                   acts[li], Ls, aff, stats)

            if DBG_STOP < 2:
                with tc.tile_pool(name="dbg", bufs=1) as dp:
                    z = dp.tile([128, E], F32, tag="z")
                    nc.vector.memset(z[:], 0.0)
                    zi = dp.tile([128, 1], U32, tag="zi")
                    nc.vector.memset(zi[:], 0)
                    for f0 in range(0, S, 128):
                        m = min(128, S - f0)
                        nc.sync.dma_start(emb_out[f0:f0 + m, :], z[:m, :])
                        nc.sync.dma_start(idx_out[f0:f0 + m, :], zi[:m, :])
                return
            with tc.tile_pool(name="featp", bufs=1) as fpool:
                fh, fl = [], []
                for eb in range(12):
                    fh.append(fpool.tile([128, S + (S & 1)], F32R,
                                         tag=f"fh{eb}", name=f"fh{eb}"))
                    fl.append(fpool.tile([128, S + (S & 1)], F32R,
                                         tag=f"fl{eb}", name=f"fl{eb}"))
                with tc.tile_pool(name="projp", bufs=1) as jp, \
                     tc.tile_pool(name="jpsum", bufs=1, space="PSUM") as jpp:
                    proj(nc, t, jp, jpp, acts[6], Ls, aff, fh, fl)
                if DBG_STOP < 3:
                    with tc.tile_pool(name="dbg", bufs=1) as dp:
                        zi = dp.tile([128, 1], U32, tag="zi")
                        nc.vector.memset(zi[:], 0)
                        for f0 in range(0, S, 128):
                            m = min(128, S - f0)
                            nc.sync.dma_start(emb_out[f0:f0 + m, :],
                                              fh[0][:m, 0:E].bitcast(F32))
                            nc.sync.dma_start(idx_out[f0:f0 + m, :], zi[:m, :])
                    return
                with tc.tile_pool(name="vqp", bufs=1) as vp, \
                     tc.tile_pool(name="vpsum", bufs=1, space="PSUM") as vpp:
                    vq_ln(nc, t, vp, vpp, fh, fl, emb_out, idx_out, S)


def evict_with_stats(nc, cp, ps, w, s1col, s2col):
    out = cp.tile([128, NT], F32, tag="out", bufs=2, name="out")
    nc.scalar.activation(out[:, :w], ps[:, :w], AF.Identity,
                         accum_out=s1col)
    sq = cp.tile([128, NT], F32, tag="sq", bufs=1, name="sq")
    nc.scalar.activation(sq[:, :w], out[:, :w], AF.Square, accum_out=s2col)
    return out


def rstd_newton(nc, pool, v_in, v_out, parts, tagp=""):
    """v_out[:parts] = 1/sqrt(v_in[:parts]) with one Newton refinement."""
    s = pool.tile([128, 8], F32, tag=f"nw_s{tagp}", bufs=2, name="nws")
    nc.scalar.activation(s[:parts, 0:1], v_in, AF.Sqrt)
    y0 = pool.tile([128, 8], F32, tag=f"nw_y0{tagp}", bufs=2, name="nwy")
    nc.vector.reciprocal(y0[:parts, 0:1], s[:parts, 0:1])
    u = pool.tile([128, 8], F32, tag=f"nw_u{tagp}", bufs=2, name="nwu")
    nc.vector.tensor_tensor(out=u[:parts, 0:1], in0=v_in, in1=y0[:parts, 0:1],
                            op=ALU.mult)
    nc.vector.tensor_tensor(out=u[:parts, 0:1], in0=u[:parts, 0:1],
                            in1=y0[:parts, 0:1], op=ALU.mult)
    nc.vector.tensor_scalar(out=u[:parts, 0:1], in0=u[:parts, 0:1],
                            scalar1=-0.5, scalar2=1.5, op0=ALU.mult,
                            op1=ALU.add)
    nc.vector.tensor_tensor(out=v_out, in0=y0[:parts, 0:1],
                            in1=u[:parts, 0:1], op=ALU.mult)


def gn_stats_to_affine(nc, misc, cp, pp, s1buf, s2buf, n_t, blk, Lout,
                       t, aff, par):
    st = cp.tile([128, 8], F32, tag="gn_st", bufs=2, name="gnst")
    nc.vector.reduce_sum(st[:, 0:1], s1buf[:, :n_t], axis=AX.X)
    nc.vector.reduce_sum(st[:, 1:2], s2buf[:, :n_t], axis=AX.X)
    w = cp.tile([128, 8], F32, tag="gn_w", bufs=2, name="gnw")
    st2 = cp.tile([128, 8], F32, tag="gn_st2", bufs=2, name="gnst2")
    # S1' = S1 + L*b
    nc.vector.tensor_scalar(out=w[:, 1:2], in0=par[:, 0:1],
                            scalar1=float(Lout), scalar2=None, op0=ALU.mult)
    nc.vector.tensor_tensor(out=st2[:, 0:1], in0=st[:, 0:1], in1=w[:, 1:2],
                            op=ALU.add)
    # S2' = S2 + 2*b*S1 + L*b^2
    nc.vector.tensor_tensor(out=w[:, 0:1], in0=st[:, 0:1], in1=par[:, 0:1],
                            op=ALU.mult)
    nc.vector.tensor_scalar(out=w[:, 3:4], in0=w[:, 0:1], scalar1=2.0,
                            scalar2=None, op0=ALU.mult)
    nc.vector.tensor_tensor(out=w[:, 2:3], in0=w[:, 1:2], in1=par[:, 0:1],
                            op=ALU.mult)
    nc.vector.tensor_tensor(out=w[:, 2:3], in0=w[:, 2:3], in1=w[:, 3:4],
                            op=ALU.add)
    nc.vector.tensor_tensor(out=st2[:, 1:2], in0=st[:, 1:2], in1=w[:, 2:3],
                            op=ALU.add)
    # group aggregate [8, 2]
    gagg = pp.tile([8, 2], F32, tag="gn_ps", bufs=2, name="gnagg")
    nc.tensor.matmul(gagg[:], _G["sel"][:], st2[:, 0:2], start=True, stop=True)
    ga = cp.tile([8, 8], F32, tag="gn_ga", bufs=2, name="gnga")
    inv = 1.0 / (16.0 * Lout)
    nc.scalar.activation(ga[:, 0:2], gagg[:], AF.Identity, scale=inv)
    nc.vector.tensor_tensor(out=ga[:, 2:3], in0=ga[:, 0:1], in1=ga[:, 0:1],
                            op=ALU.mult)
    nc.vector.tensor_tensor(out=ga[:, 2:3], in0=ga[:, 1:2], in1=ga[:, 2:3],
                            op=ALU.subtract)
    nc.vector.tensor_tensor(out=ga[:, 2:3], in0=ga[:, 2:3],
                            in1=_G["eps"][0:8, 0:1], op=ALU.add)
    rstd_newton(nc, cp, ga[:, 2:3], ga[:, 3:4], 8, tagp="g")
    mr = cp.tile([8, 8], F32, tag="gn_mr", bufs=2, name="gnmr")
    nc.vector.tensor_copy(mr[:, 0:1], ga[:, 0:1])
    nc.vector.tensor_copy(mr[:, 1:2], ga[:, 3:4])
    gex = pp.tile([128, 2], F32, tag="gn_ps", bufs=2, name="gnexp")
    nc.tensor.matmul(gex[:], _G["selT"][:], mr[:, 0:2], start=True, stop=True)
    exs = cp.tile([128, 8], F32, tag="gn_exs", bufs=2, name="gnexs")
    nc.scalar.activation(exs[:, 0:2], gex[:], AF.Identity)
    # scale_c = gamma*rstd ; bias_c = beta + scale_c*(b - mean)
    a = misc.tile([128, 8], F32, tag=f"aff{blk}", name=f"aff{blk}")
    nc.vector.tensor_tensor(out=a[:, 0:1], in0=par[:, 1:2], in1=exs[:, 1:2],
                            op=ALU.mult)
    tmp = cp.tile([128, 8], F32, tag="gn_tmp", bufs=2, name="gntmp")
    nc.vector.tensor_tensor(out=tmp[:, 0:1], in0=par[:, 0:1], in1=exs[:, 0:1],
                            op=ALU.subtract)
    nc.vector.tensor_tensor(out=tmp[:, 0:1], in0=tmp[:, 0:1], in1=a[:, 0:1],
                            op=ALU.mult)
    nc.vector.tensor_tensor(out=a[:, 1:2], in0=par[:, 2:3], in1=tmp[:, 0:1],
                            op=ALU.add)
    aff[blk] = a


def conv0(nc, t, misc, cp, pp, a0, Ls, aff, stats):
    L1 = Ls[1]
    n_t = _cdiv(L1, NT)
    wh = cp.tile([10, 256], F32R, tag="w0h", name="w0h")
    nc.sync.dma_start(wh[:], t["w0h"][:])
    wl = cp.tile([10, 256], F32R, tag="w0l", name="w0l")
    nc.sync.dma_start(wl[:], t["w0l"][:])
    s1 = {}
    s2 = {}
    for b in range(2):
        s1[b] = cp.tile([128, (n_t + 7) // 8 * 8], F32, tag=f"s1_{b}", bufs=2, name=f"s1c0{b}")
        s2[b] = cp.tile([128, (n_t + 7) // 8 * 8], F32, tag=f"s2_{b}", bufs=2, name=f"s2c0{b}")
    XG = 3
    for tg in range(0, n_t, XG):
        g0c = tg * NT
        gcols = min(XG * NT, L1 - g0c)
        gcols2 = gcols + (gcols & 1)
        xh = cp.tile([10, XG * NT], F32R, tag="x0h", bufs=2, name="x0h")
        nc.sync.dma_start(xh[:, :gcols], t["x0h"][:, g0c:g0c + gcols])
        xl = cp.tile([10, XG * NT], F32R, tag="x0l", bufs=2, name="x0l")
        nc.sync.dma_start(xl[:, :gcols], t["x0l"][:, g0c:g0c + gcols])
        if gcols2 > gcols:
            nc.vector.memset(xh[:, gcols:gcols2].bitcast(F32), 0.0)
            nc.vector.memset(xl[:, gcols:gcols2].bitcast(F32), 0.0)
        for ti in range(tg, min(tg + XG, n_t)):
            n0 = ti * NT
            w = min(NT, L1 - n0)
            w2 = w + (w & 1)
            c0 = n0 - g0c
            for b in range(2):
                ps = pp.tile([128, NT], F32, tag=f"mm{b}", bufs=3,
                             name="psc0")
                co = slice(b * 128, (b + 1) * 128)
                nc.tensor.matmul(ps[:, :w2], wh[:, co], xh[:, c0:c0 + w2],
                                 start=True, stop=False)
                nc.tensor.matmul(ps[:, :w2], wh[:, co], xl[:, c0:c0 + w2],
                                 start=False, stop=False)
                nc.tensor.matmul(ps[:, :w2], wl[:, co], xh[:, c0:c0 + w2],
                                 start=False, stop=True)
                out = evict_with_stats(nc, cp, ps, w, s1[b][:, ti:ti + 1],
                                       s2[b][:, ti:ti + 1])
                nc.sync.dma_start(a0[b * 128:(b + 1) * 128, n0:n0 + w],
                                  out[:, :w])
    for b in range(2):
        gn_stats_to_affine(nc, misc, cp, pp, s1[b], s2[b], n_t, b, L1,
                           t, aff, _G[f"par0_{b}"])


def conv_layer(nc, t, misc, cp, pp, li, a_in, a_out, Ls, aff, stats):
    Lin, Lout = Ls[li], Ls[li + 1]
    cin = 256 if li == 1 else 512
    n_ci = cin // 128
    n_t = _cdiv(Lout, NT)
    WMAX = 2 * NT + 10
    # snapshot input affines (written by previous layer) before this layer
    # overwrites aff[blk] at its own finalize
    in_aff = [aff[ci] for ci in range(n_ci)]
    for pas in range(2):
        blocks = [pas * 2, pas * 2 + 1]
        whl = {}
        for b in blocks:
            for hl in "hl":
                wt = cp.tile([128, n_ci * 10 * 128], F32R,
                             tag=f"w{hl}{b % 2}", name=f"w{hl}{b % 2}")
                for ci in range(n_ci):
                    for k in range(10):
                        co = ((ci * 10) + k) * 128
                        nc.sync.dma_start(
                            wt[:, co:co + 128],
                            t[f"w{li}{hl}"][k, ci * 128:(ci + 1) * 128,
                                            b * 128:(b + 1) * 128])
                whl[(b, hl)] = wt
        sb = {}
        for b in blocks:
            sb[("s1", b)] = cp.tile([128, (n_t + 7) // 8 * 8], F32, tag=f"s1_{b % 2}",
                                    bufs=2, name=f"s1_{li}_{b}")
            sb[("s2", b)] = cp.tile([128, (n_t + 7) // 8 * 8], F32, tag=f"s2_{b % 2}",
                                    bufs=2, name=f"s2_{li}_{b}")
        for ti in range(n_t):
            n0 = ti * NT
            w = min(NT, Lout - n0)
            w2 = w + (w & 1)
            ilo = 2 * n0 - 5
            width2 = 2 * w2 + 8
            vlo, vhi = max(ilo, 0), min(ilo + width2 - 1, Lin - 1)
            ds, de = vlo - ilo, vhi - ilo + 1
            his, los = [], []
            PW = NT + 4
            for ci in range(n_ci):
                raw = cp.tile([128, WMAX], F32, tag="raw", bufs=3, name="raw")
                nc.sync.dma_start(raw[:, ds:de],
                                  a_in[ci * 128:(ci + 1) * 128, vlo:vhi + 1])
                ac = in_aff[ci]
                # deinterleave into even/odd input phases so conv matmul rhs
                # APs are stride-1 (stride-2 fp32r matmul runs at half rate):
                # even phase e0 = n0-2 at raw col 2r+1; odd phase o0 = n0-3
                # at raw col 2r.
                phs = []
                for po, (j0, rs, re) in enumerate((
                        (1, ds // 2, de // 2),
                        (0, (ds + 1) // 2, (de + 1) // 2))):
                    gph = cp.tile([128, PW], F32, tag=f"g{po}", bufs=2,
                                  name=f"g{po}")
                    nc.scalar.activation(gph[:, rs:re],
                                         raw[:, j0 + 2 * rs:j0 + 2 * re:2],
                                         AF.Gelu, bias=ac[:, 1:2],
                                         scale=ac[:, 0:1])
                    hi = cp.tile([128, PW], F32R, tag=f"xh{po}_{ci}", bufs=2,
                                 name=f"xh{po}_{ci}")
                    lo = cp.tile([128, PW], F32R, tag=f"xl{po}_{ci}", bufs=2,
                                 name=f"xl{po}_{ci}")
                    nc.vector.tensor_copy(hi[:, rs:re], gph[:, rs:re])
                    nc.vector.tensor_tensor(out=lo[:, rs:re],
                                            in0=gph[:, rs:re],
                                            in1=hi[:, rs:re].bitcast(F32),
                                            op=ALU.subtract)
                    wph = w2 + 4
                    if rs > 0:
                        nc.vector.memset(hi[:, 0:rs].bitcast(F32), 0.0)
                        nc.vector.memset(lo[:, 0:rs].bitcast(F32), 0.0)
                    if re < wph:
                        nc.vector.memset(hi[:, re:wph].bitcast(F32), 0.0)
                        nc.vector.memset(lo[:, re:wph].bitcast(F32), 0.0)
                    phs.append((hi, lo))
                his.append((phs[0][0], phs[1][0]))   # (even, odd) hi
                los.append((phs[0][1], phs[1][1]))   # (even, odd) lo
            for b in blocks:
                ps = pp.tile([128, NT], F32, tag=f"mm{b % 2}", bufs=3,
                             name="psc")
                first = True
                for ci in range(n_ci):
                    wh_ci = whl[(b, 'h')]
                    wl_ci = whl[(b, 'l')]
                    for k in range(10):
                        co = ((ci * 10) + k) * 128
                        if k % 2 == 1:
                            st = (k - 1) // 2
                            rh = his[ci][0][:, st:st + w2]
                            rl = los[ci][0][:, st:st + w2]
                        else:
                            st = k // 2
                            rh = his[ci][1][:, st:st + w2]
                            rl = los[ci][1][:, st:st + w2]
                        last = (ci == n_ci - 1 and k == 9)
                        nc.tensor.matmul(ps[:, :w2], wh_ci[:, co:co + 128], rh,
                                         start=first, stop=False)
                        first = False
                        nc.tensor.matmul(ps[:, :w2], wh_ci[:, co:co + 128], rl,
                                         start=False, stop=False)
                        nc.tensor.matmul(ps[:, :w2], wl_ci[:, co:co + 128], rh,
                                         start=False, stop=last)
                out = evict_with_stats(nc, cp, ps, w,
                                       sb[("s1", b)][:, ti:ti + 1],
                                       sb[("s2", b)][:, ti:ti + 1])
                nc.sync.dma_start(a_out[b * 128:(b + 1) * 128, n0:n0 + w],
                                  out[:, :w])
        for b in blocks:
            gn_stats_to_affine(nc, misc, cp, pp, sb[("s1", b)],
                               sb[("s2", b)], n_t, b, Lout, t, aff,
                               _G[f"par{li}_{b}"])


def proj(nc, t, jp, jpp, a6, Ls, aff, fh, fl):
    S = Ls[7]
    S2 = S + (S & 1)
    a6h, a6l = [], []
    for ci in range(4):
        raw = jp.tile([128, S], F32, tag="a6raw", bufs=2, name="a6raw")
        nc.sync.dma_start(raw[:], a6[ci * 128:(ci + 1) * 128, :])
        g = jp.tile([128, S], F32, tag="a6g", bufs=2, name="a6g")
        ac = aff[ci]
        nc.scalar.activation(g[:], raw[:], AF.Gelu, bias=ac[:, 1:2],
                             scale=ac[:, 0:1])
        hi = jp.tile([128, S2], F32R, tag=f"a6h{ci}", name=f"a6h{ci}")
        lo = jp.tile([128, S2], F32R, tag=f"a6l{ci}", name=f"a6l{ci}")
        nc.vector.tensor_copy(hi[:, :S], g[:])
        nc.vector.tensor_tensor(out=lo[:, :S], in0=g[:],
                                in1=hi[:, :S].bitcast(F32), op=ALU.subtract)
        if S2 > S:
            nc.vector.memset(hi[:, S:S2].bitcast(F32), 0.0)
            nc.vector.memset(lo[:, S:S2].bitcast(F32), 0.0)
        a6h.append(hi)
        a6l.append(lo)
    n_t = _cdiv(S, NT)
    for eb in range(12):
        pwh = jp.tile([128, 4 * 128], F32R, tag="pwh", bufs=2, name="pwh")
        pwl = jp.tile([128, 4 * 128], F32R, tag="pwl", bufs=2, name="pwl")
        for ci in range(4):
            nc.sync.dma_start(pwh[:, ci * 128:(ci + 1) * 128],
                              t["pwh"][ci * 128:(ci + 1) * 128,
                                       eb * 128:(eb + 1) * 128])
            nc.sync.dma_start(pwl[:, ci * 128:(ci + 1) * 128],
                              t["pwl"][ci * 128:(ci + 1) * 128,
                                       eb * 128:(eb + 1) * 128])
        pbias = jp.tile([128, 8], F32, tag="pbias", bufs=2, name="pbias")
        nc.sync.dma_start(
            pbias[:, 0:1],
            t["pb"][eb * 128:(eb + 1) * 128].rearrange("(p f) -> p f", f=1))
        fraw = jp.tile([128, S], F32, tag="fraw", bufs=2, name="fraw")
        for ti in range(n_t):
            n0 = ti * NT
            w = min(NT, S - n0)
            w2 = w + (w & 1)
            ps = jpp.tile([128, NT], F32, tag=f"pp{ti % 2}", bufs=2,
                          name="psp")
            first = True
            for ci in range(4):
                cw = slice(ci * 128, (ci + 1) * 128)
                last = ci == 3
                nc.tensor.matmul(ps[:, :w2], pwh[:, cw],
                                 a6h[ci][:, n0:n0 + w2],
                                 start=first, stop=False)
                first = False
                nc.tensor.matmul(ps[:, :w2], pwh[:, cw],
                                 a6l[ci][:, n0:n0 + w2],
                                 start=False, stop=False)
                nc.tensor.matmul(ps[:, :w2], pwl[:, cw],
                                 a6h[ci][:, n0:n0 + w2],
                                 start=False, stop=last)
            # (eviction below only covers the true w columns)
            nc.vector.tensor_scalar(out=fraw[:, n0:n0 + w], in0=ps[:, :w],
                                    scalar1=pbias[:, 0:1], scalar2=None,
                                    op0=ALU.add)
        nc.vector.tensor_copy(fh[eb][:, :S], fraw[:])
        nc.vector.tensor_tensor(out=fl[eb][:, :S], in0=fraw[:],
                                in1=fh[eb][:, :S].bitcast(F32),
                                op=ALU.subtract)
        if S2 > S:
            nc.vector.memset(fh[eb][:, S:S2].bitcast(F32), 0.0)
            nc.vector.memset(fl[eb][:, S:S2].bitcast(F32), 0.0)


def vq_ln(nc, t, vp, vpp, fh, fl, emb_out, idx_out, S):
    S2 = S + (S & 1)
    csq = vp.tile([1, NCODE], F32, tag="csq1", name="csq1")
    nc.sync.dma_start(csq[:], t["csqn"][:])
    csqb = vp.tile([128, NCODE], F32, tag="csqb", name="csqb")
    if DBG_NOPB:
        nc.vector.memset(csqb[:], 0.0)
    else:
        nc.gpsimd.partition_broadcast(csqb[:], csq[:])
    modb = vp.tile([128, E], F32, tag="modb", name="modb")
    lngb = vp.tile([128, E], F32, tag="lngb", name="lngb")
    lnbb = vp.tile([128, E], F32, tag="lnbb", name="lnbb")
    for bt, nm in ((modb, "modality"), (lngb, "lng"), (lnbb, "lnb")):
        if DBG_NOPB:
            nc.vector.memset(bt[:], 0.0)
            continue
        m1 = vp.tile([1, E], F32, tag="m1", bufs=2, name="m1")
        nc.sync.dma_start(m1[:], t[nm][:])
        nc.gpsimd.partition_broadcast(bt[:], m1[:])

    nfb = _cdiv(S, FB)
    for fbg in range(0, nfb, 4):
        fbs = list(range(fbg, min(fbg + 4, nfb)))
        pss = {}
        for fb in fbs:
            pss[fb] = vpp.tile([128, NCODE], F32, tag=f"vq{fb % 4}", bufs=1,
                               name=f"psv{fb % 4}")
        for eb in range(12):
            ch = vp.tile([128, NCODE], F32R, tag="cbh", bufs=2, name="cbh")
            cl = vp.tile([128, NCODE], F32R, tag="cbl", bufs=2, name="cbl")
            nc.sync.dma_start(ch[:], t["cbh"][eb * 128:(eb + 1) * 128, :])
            nc.sync.dma_start(cl[:], t["cbl"][eb * 128:(eb + 1) * 128, :])
            for fb in fbs:
                f0 = fb * FB
                m = min(FB, S - f0)
                m2 = m + (m & 1)
                ps = pss[fb]
                for half in range(2):
                    cs = slice(half * 512, (half + 1) * 512)
                    nc.tensor.matmul(ps[:m2, cs], fh[eb][:, f0:f0 + m2],
                                     ch[:, cs], start=(eb == 0), stop=False)
                    nc.tensor.matmul(ps[:m2, cs], fh[eb][:, f0:f0 + m2],
                                     cl[:, cs], start=False, stop=False)
                    nc.tensor.matmul(ps[:m2, cs], fl[eb][:, f0:f0 + m2],
                                     ch[:, cs], start=False,
                                     stop=(eb == 11))
        vq_post(nc, t, vp, fh, fl, emb_out, idx_out, S, fbs, pss,
                csqb, modb, lngb, lnbb)


def vq_post(nc, t, vp, fh, fl, emb_out, idx_out, S, fbs, pss,
            csqb, modb, lngb, lnbb):
    for fb in fbs:
        f0 = fb * FB
        m = min(FB, S - f0)
        ps = pss[fb]
        score = vp.tile([128, NCODE], F32, tag="score", bufs=3, name="score")
        nc.vector.tensor_tensor(out=score[:m, :], in0=ps[:m, :],
                                in1=csqb[:m, :], op=ALU.add)
        mx8 = vp.tile([128, 8], F32, tag="mx8", bufs=3, name="mx8")
        ix8 = vp.tile([128, 8], U32, tag="ix8", bufs=3, name="ix8")
        nc.vector.max(mx8[:m, :], score[:m, :])
        nc.vector.max_index(ix8[:m, :], mx8[:m, :], score[:m, :])
        nc.sync.dma_start(idx_out[f0:f0 + m, :], ix8[:m, 0:1])
        q = vp.tile([128, E], F32, tag="q", bufs=3, name="q")
        if DBG_NOGATHER:
            nc.sync.dma_start(q[:m, :], t["codebook"][f0:f0 + m, :])
        else:
            nc.gpsimd.indirect_dma_start(
                out=q[:m, :], out_offset=None, in_=t["codebook"][:],
                in_offset=bass.IndirectOffsetOnAxis(ap=ix8[:m, 0:1], axis=0))
        pos = vp.tile([128, E], F32, tag="pos", bufs=2, name="pos")
        nc.sync.dma_start(pos[:m, :], t["pos"][f0:f0 + m, :])
        nc.vector.tensor_tensor(out=q[:m, :], in0=q[:m, :], in1=pos[:m, :],
                                op=ALU.add)
        nc.vector.tensor_tensor(out=q[:m, :], in0=q[:m, :], in1=modb[:m, :],
                                op=ALU.add)
        # LayerNorm over E
        stat = vp.tile([128, 8], F32, tag="lnstat", bufs=3, name="lnstat")
        nc.vector.reduce_sum(stat[:m, 0:1], q[:m, :], axis=AX.X)
        nc.vector.tensor_scalar(out=stat[:m, 1:2], in0=stat[:m, 0:1],
                                scalar1=1.0 / E, scalar2=None, op0=ALU.mult)
        nc.vector.tensor_scalar(out=q[:m, :], in0=q[:m, :],
                                scalar1=stat[:m, 1:2], scalar2=None,
                                op0=ALU.subtract)
        sq = vp.tile([128, E], F32, tag="lnsq", bufs=2, name="lnsq")
        nc.scalar.activation(sq[:m, :], q[:m, :], AF.Square,
                             accum_out=stat[:m, 2:3])
        v = vp.tile([128, 8], F32, tag="lnv", bufs=3, name="lnv")
        nc.vector.tensor_scalar(out=v[:m, 0:1], in0=stat[:m, 2:3],
                                scalar1=1.0 / E, scalar2=None, op0=ALU.mult)
        nc.vector.tensor_tensor(out=v[:m, 0:1], in0=v[:m, 0:1],
                                in1=_G["eps"][:m, 0:1], op=ALU.add)
        rstd_newton(nc, vp, v[:m, 0:1], v[:m, 1:2], m, tagp="v")
        nc.vector.tensor_scalar(out=q[:m, :], in0=q[:m, :],
                                scalar1=v[:m, 1:2], scalar2=None,
                                op0=ALU.mult)
        nc.vector.tensor_tensor(out=q[:m, :], in0=q[:m, :], in1=lngb[:m, :],
                                op=ALU.mult)
        nc.vector.tensor_tensor(out=q[:m, :], in0=q[:m, :], in1=lnbb[:m, :],
                                op=ALU.add)
        nc.sync.dma_start(emb_out[f0:f0 + m, :], q[:m, :])


# ---------------------------------------------------------------- host side

def tf32_rne(x):
    b = np.ascontiguousarray(x, dtype=np.float32).view(np.uint32)
    keep = np.uint32(13)
    rb = np.uint32(1 << 12)
    low = b & np.uint32((1 << 13) - 1)
    b2 = b & ~np.uint32((1 << 13) - 1)
    inc = (low > rb) | ((low == rb) & (((b2 >> keep) & np.uint32(1)) == 1))
    return (b2 + (inc.astype(np.uint32) << keep)).view(np.float32)


def split_hl(x):
    x = np.ascontiguousarray(x, np.float32)
    h = tf32_rne(x)
    l = tf32_rne(x - h)
    return h, l


def prep_shared(inputs, L0):
    Ls = layer_dims(L0)
    S = Ls[7]
    m = {}
    w0 = np.asarray(inputs["conv_w0"], np.float32)[:, 0, :].T
    m["w0h"], m["w0l"] = split_hl(w0)
    m["b0"] = np.ascontiguousarray(inputs["conv_b0"], np.float32)
    m["g0"] = np.ascontiguousarray(inputs["gn_g0"], np.float32)
    m["be0"] = np.ascontiguousarray(inputs["gn_b0"], np.float32)
    w1 = np.asarray(inputs["conv_w1"], np.float32).transpose(2, 1, 0)
    m["w1h"], m["w1l"] = split_hl(w1)
    m["b1"] = np.ascontiguousarray(inputs["conv_b1"], np.float32)
    gn_gr = np.asarray(inputs["gn_gr"], np.float32)
    gn_br = np.asarray(inputs["gn_br"], np.float32)
    m["g1"] = np.ascontiguousarray(gn_gr[0])
    m["be1"] = np.ascontiguousarray(gn_br[0])
    wr = np.asarray(inputs["conv_wr"], np.float32)
    br = np.asarray(inputs["conv_br"], np.float32)
    for i in range(5):
        m[f"w{i + 2}h"], m[f"w{i + 2}l"] = split_hl(wr[i].transpose(2, 1, 0))
        m[f"b{i + 2}"] = np.ascontiguousarray(br[i])
        m[f"g{i + 2}"] = np.ascontiguousarray(gn_gr[i + 1])
        m[f"be{i + 2}"] = np.ascontiguousarray(gn_br[i + 1])
    pw = np.asarray(inputs["proj_w"], np.float32).T
    m["pwh"], m["pwl"] = split_hl(pw)
    m["pb"] = np.ascontiguousarray(inputs["proj_b"], np.float32)
    cb = np.ascontiguousarray(inputs["codebook"], np.float32)
    m["cbh"], m["cbl"] = split_hl((2.0 * cb).T)
    m["csqn"] = np.ascontiguousarray(
        -np.sum(cb.astype(np.float64) ** 2, axis=1).astype(np.float32)[None, :])
    m["codebook"] = cb
    m["pos"] = np.ascontiguousarray(
        np.asarray(inputs["pos_enc"], np.float32)[0, :S, :])
    m["modality"] = np.ascontiguousarray(
        np.asarray(inputs["modality"], np.float32).reshape(1, E))
    m["lng"] = np.ascontiguousarray(
        np.asarray(inputs["ln_g"], np.float32).reshape(1, E))
    m["lnb"] = np.ascontiguousarray(
        np.asarray(inputs["ln_b"], np.float32).reshape(1, E))
    sel = np.zeros((128, 8), np.float32)
    sel[np.arange(128), np.arange(128) // 16] = 1.0
    m["sel"] = sel
    m["selT"] = np.ascontiguousarray(sel.T)
    return m


def prep_waveform(wav, L0):
    L1 = layer_dims(L0)[1]
    pad = np.zeros(L0 + 10, np.float32)
    pad[5:5 + L0] = wav
    idx = np.arange(L1)[None, :] * 5 + np.arange(10)[:, None]
    h, l = split_hl(pad[idx])
    return {"x0h": h, "x0l": l}


_NC_CACHE = {}


def get_nc(L0):
    if L0 not in _NC_CACHE:
        _NC_CACHE[L0] = build_nc(L0)
    return _NC_CACHE[L0]


def make_in_maps(inputs):
    wav = np.asarray(inputs["waveform"], np.float32)
    B, L0 = wav.shape
    shared = prep_shared(inputs, L0)
    in_maps = []
    for b in range(B):
        im = dict(shared)
        im.update(prep_waveform(wav[b], L0))
        in_maps.append(im)
    return in_maps


def kernel(**inputs):
    from concourse.bass_utils import run_bass_kernel_spmd

    wav = np.asarray(inputs["waveform"], np.float32)
    B, L0 = wav.shape
    assert B == N_CORES
    S = layer_dims(L0)[7]
    nc = get_nc(L0)
    in_maps = make_in_maps(inputs)
    res = run_bass_kernel_spmd(nc, in_maps, list(range(N_CORES))).results
    emb = np.stack([res[b]["emb"] for b in range(B)])
    idx = np.stack([res[b]["idx"][:, 0] for b in range(B)]).astype(np.int32)
    mask = np.ones((B, S), np.int32)
    return emb, mask, idx
